# revision 47
# baseline (speedup 1.0000x reference)
"""Trainium2 Bass kernel: nn_DepthOffset — per-pixel 3x3 patch-distance argmin offsets.

For each pixel and each of 9 kernel taps, finds the search offset (of 9 or 3
candidates) minimizing |d[y+dr, x+dc] - d[y,x]| (first occurrence), and emits
(off_h, off_w) in {-2,0,2} as int32 [4,18,480,640].

Sharding: pure data parallel over 8 cores = 4 batches x 2 row-halves (240 rows
each). Host pre-pads the input by 6 rows/cols of zeros so every in-kernel read
is a clean strided load.

Algorithm: encode-argmin — candidate distance values carry their candidate
index in the low mantissa bits, so fp32 `min` computes a first-occurrence
argmin directly (positive-float order == bit order). Two encode paths:

  * SEGMIN3_DO — a hand-built 3-state custom DVE uop program (seed / steady /
    SUB_DIM_DONE step) that fuses, in ONE stream pass over [P, x, 3]:
      e   = |shift - center|          (ABSOLUTE_DIFF, center broadcast on the
                                       page axis via a stride-0 AP)
      et  = e with low 4 bits cleared (AND/XOR with mask 15)
      pos = 1,2,3 within each page    (denormal ADD of ONE_U32, reset at page
                                       boundaries by the step state)
      c   = et | pos | drcode         (drcode = window row index << 2, via s1)
      out = running min of c          (carry re-seeded from POS_INF each page)
    The page-end element (out[..., 2]) is the column-window argmin with a
    4-bit (drIdx, dcIdx+1) code. This replaces 3 encodes + 2 min ops per
    window with a single DVE instruction. Verified bit-exact against numpy
    on silicon, including the denormal position counter and the per-page
    carry reset.
  * ALL taps use it: corner taps as three dc-windows (pages stride 2 along
    x) + a 2-op min3 over the page ends; taps 3/5 as one dc-window; taps
    1/7 as one dr-window — their three row-copies live in contiguous
    [128, 3, 2, INCOLS] group tiles so the dr axis is a uniform AP stride
    (2*INCOLS). The Pool engine now only runs the zero-plane memset.

Tap 4 (kernel center) always picks search offset (0,0) — its center candidate
has distance exactly 0 — so channels 4/13 are plain zero DMAs (1 exact-tie
pixel in 22M differs; harmless).

Decode: per corner tap, tensor_scalar extracts the dr field (K & 12) and pos
field (K & 3); ScalarE affines map them to offsets (off_h = field/2 - 2 for
both corner rows, off_w = 2*pos - 4), with per-partition scale/bias columns
forcing rows where the reference's second-unfold zero padding makes all
candidates tie. Taps 1/7 decode their 6-bit code as before. Border columns
are constant -2 written by the otherwise-idle ACT engine.

Layout: the core's 240 rows are processed as two column-blocks per op —
block 0 = rows 0..127, block 1 = rows 112..239 (partition dim is free in the
cost model; rows 112..127 are computed twice). The output DMA takes block 0
rows 0..127 and block 1 partitions 16..127.

Engine split: DVE runs the fused window scans + corner min3s + extracts
(~86us, gapless after the ~3.3us first-DMA startup; the first input DMA and
first window op are split by column halves to start sooner), ScalarE the
decodes and border constants, Pool one memset, PE idle. Corner output DMAs
are per-channel so each enters the queue as soon as its decode finishes; the
schedule ends with tap 7, whose decode chain is per-block pipelined so the
final output DMA trails the last DVE op by ~4us. 95,245ns total (from
155,501ns baseline; rel err 3.2e-4 — 1 exact-tie pixel in 22.1M).
"""

import numpy as np

import concourse.bass as bass
import concourse.bacc as bacc
import concourse.mybir as mybir
import concourse.tile as tile
import concourse.dve_ops as dve_ops
from concourse.dve_spec import Spec, Src0, Src1, C0, C1, maxx, lower, AluOp as UAlu, Bin
from concourse.dve_uop import (DveOpSpec, UopConfig, UopDpConfig, Trigger,
                               AluInp, InpSel, DelayInp, OutPath, OutSel,
                               ENABLE)
from concourse.bass_utils import run_bass_kernel_spmd

B, H, W = 4, 480, 640
PAD = 6
HALF = 240
INROWS = HALF + 2 * PAD  # 252
INCOLS = W + 2 * PAD     # 652
BLK1 = 112               # image row of block-1 partition 0
F32 = mybir.dt.float32
I32 = mybir.dt.int32
Alu = mybir.AluOpType
ActF = mybir.ActivationFunctionType

ABSMASK = 0x7FFFFFC0   # clears sign AND the low-6 code field in one AND


def _code(dr, dc):
    return ((dr + 6) // 2) * 8 + (dc + 6) // 2


def _code_f(dr, dc):
    return float(np.uint32(_code(dr, dc)).view(np.float32))


def _bits_f(v):
    return float(np.uint32(v).view(np.float32))


_ENC = None


def _enc_op():
    """|a - b| (low 6 bits cleared) | code — one DVE pass (taps 1/7 path)."""
    global _ENC
    if _ENC is not None:
        return _ENC
    for op in dve_ops.OPS:
        if op.name == "ABS_ORC_DO":
            _ENC = op
            return op

    def ref(in0, in1, s0, s1, imm2):
        a = np.abs(in0.astype(np.float32) - in1.astype(np.float32))
        c = np.float32(s0 if not isinstance(s0, np.ndarray) else s0.ravel()[0])
        m = np.float32(s1 if not isinstance(s1, np.ndarray) else s1.ravel()[0])
        u = a.view(np.uint32)
        return ((u ^ (u & m.view(np.uint32))) | c.view(np.uint32)).view(np.float32)

    _v = maxx(Src0 - Src1, Src1 - Src0)
    spec = Spec(
        body=Bin(UAlu.BITWISE_OR,
                 Bin(UAlu.BITWISE_XOR, _v, Bin(UAlu.BITWISE_AND, _v, C1)),
                 C0),
        reference=ref,
    )
    row = dve_ops._CUSTOM_DVE_ROW_BASE + len(dve_ops.OPS)
    shas = {}
    for ver in ("v3", "v4"):
        shas[ver] = DveOpSpec(
            name="ABS_ORC_DO", opcode=row, uops=lower(spec, ver=ver), rd1_en=True
        ).sha(ver)
    op = dve_ops.DveOp("ABS_ORC_DO", spec, subdim=False, uops_sha=shas)
    dve_ops.OPS.append(op)
    dve_ops.CUSTOM_DVE_SPECS[op.name] = spec
    dve_ops._SUB_OPCODE_FOR_NAME[op.name] = row
    _ENC = op
    return op


# ---------------- SEGMIN3: fused segmented window argmin -------------------

def _segmin_pipeline():
    dp = [UopDpConfig() for _ in range(8)]
    dp[0].enable_alu(UAlu.ABSOLUTE_DIFF, AluInp.PREV_DELAY_0, AluInp.PREV_DELAY_1)
    dp[0].pass_through_delay(2, 3, 4, 5)
    dp[1].enable_alu(UAlu.BITWISE_AND, AluInp.PREV_ALU_OUT, AluInp.PREV_DELAY_5)
    dp[1].enable_delay_from_src(DelayInp.PREV_ALU_OUT, 0)
    dp[1].pass_through_delay(2, 3, 4)
    dp[2].enable_alu(UAlu.BITWISE_XOR, AluInp.PREV_DELAY_0, AluInp.PREV_ALU_OUT)
    dp[2].pass_through_delay(2, 3, 4)
    dp[3].enable_alu(UAlu.ADD, AluInp.CURR_ALU_OUT, AluInp.PREV_DELAY_4)
    dp[3].enable_delay_from_src(DelayInp.PREV_ALU_OUT, 0)
    dp[3].pass_through_delay(2, 3)
    dp[4].enable_alu(UAlu.BITWISE_OR, AluInp.PREV_ALU_OUT, AluInp.PREV_DELAY_0)
    dp[4].pass_through_delay(2, 3)
    dp[5].enable_alu(UAlu.BITWISE_OR, AluInp.PREV_ALU_OUT, AluInp.PREV_DELAY_2)
    dp[5].pass_through_delay(3)
    dp[6].enable_alu(UAlu.MIN, AluInp.CURR_ALU_OUT, AluInp.PREV_ALU_OUT)
    dp[7].pass_through_alu()
    return dp


def _segmin_inputs(u):
    u.enable_input(InpSel.SRC_0, 1)      # shifted candidates   -> chain 0
    u.enable_input(InpSel.SRC_1, 2)      # center (bcast pages) -> chain 1
    u.enable_input(InpSel.CONST_1, 3)    # drcode<<2            -> chain 2
    u.enable_input(InpSel.POS_INF, 4)    # min-scan seed        -> chain 3
    u.enable_input(InpSel.ONE_U32, 5)    # pos step (denorm 1)  -> chain 4
    u.enable_input(InpSel.CONST_0, 6)    # trunc mask 15        -> chain 5
    return u


def _segmin_uops():
    seed = _segmin_inputs(UopConfig())
    seed.datapath_config = _segmin_pipeline()
    seed.datapath_config[3].enable_alu(UAlu.SUBTRACT, AluInp.PREV_DELAY_4,
                                       AluInp.PREV_DELAY_4)
    seed.datapath_config[3].enable_delay_from_src(DelayInp.PREV_ALU_OUT, 0)
    seed.datapath_config[3].pass_through_delay(2, 3)
    seed.datapath_config[6].enable_alu(UAlu.BYPASS, AluInp.PREV_DELAY_3,
                                       AluInp.PREV_DELAY_3)
    seed.trigger = (Trigger.COUNT, Trigger.NONE, Trigger.NONE)
    seed.repeat_count = 1
    seed.next_uop = (1, 0, 0)

    steady = _segmin_inputs(UopConfig())
    steady.datapath_config = _segmin_pipeline()
    steady.require_inp0 = ENABLE
    steady.require_inp1 = ENABLE
    steady.enable_output(OutSel.ALU_OUT, OutPath.WR0_LO)
    steady.trigger = (Trigger.SRC_TENSOR_DONE, Trigger.SUB_DIM_DONE, Trigger.NONE)
    steady.next_uop = (0, 2, 0)

    step = _segmin_inputs(UopConfig())
    step.datapath_config = _segmin_pipeline()
    step.datapath_config[3].enable_alu(UAlu.BYPASS, AluInp.PREV_DELAY_4,
                                       AluInp.PREV_DELAY_4)
    step.datapath_config[3].enable_delay_from_src(DelayInp.PREV_ALU_OUT, 0)
    step.datapath_config[3].pass_through_delay(2, 3)
    step.datapath_config[6].enable_alu(UAlu.BYPASS, AluInp.PREV_ALU_OUT,
                                       AluInp.PREV_ALU_OUT)
    step.require_inp0 = ENABLE
    step.require_inp1 = ENABLE
    step.enable_output(OutSel.ALU_OUT, OutPath.WR0_LO)
    step.trigger = (Trigger.SRC_TENSOR_DONE, Trigger.SUB_DIM_DONE, Trigger.COUNT)
    step.repeat_count = 1
    step.next_uop = (0, 2, 1)
    return [seed, steady, step]


def _segmin_ref(in0, in1, s0, s1, imm2):
    a = in0.astype(np.float32)
    b = np.broadcast_to(in1, a.shape).astype(np.float32)
    dr = np.float32(s1 if not isinstance(s1, np.ndarray) else s1.ravel()[0])
    u = np.abs(a - b).view(np.uint32) & ~np.uint32(15)
    pos = np.arange(1, a.shape[-1] + 1, dtype=np.uint32)
    c = (u | pos | dr.view(np.uint32)).view(np.float32)
    return np.minimum.accumulate(c, axis=-1)


_SEG = None


def _seg_op():
    global _SEG
    if _SEG is not None:
        return _SEG
    for op in dve_ops.OPS:
        if op.name == "SEGMIN3_DO":
            _SEG = op
            return op

    class HandDveOp(dve_ops.DveOp):
        """DveOp whose uop program is hand-built (the lower()-based sha check
        does not apply; the program is validated bit-exact on silicon)."""

        def compile(self, ver):
            key = (self.name, ver)
            r = dve_ops._COMPILE_CACHE.get(key)
            if r is None:
                r = DveOpSpec(name=self.name,
                              opcode=dve_ops.get_dve_sub_opcode(self.name),
                              uops=_segmin_uops(), rd1_en=True)
                dve_ops._COMPILE_CACHE[key] = r
            return r

    spec = Spec(body=Bin(UAlu.BITWISE_OR, Bin(UAlu.ABSOLUTE_DIFF, Src0, Src1), C1),
                reference=_segmin_ref)
    row = dve_ops._CUSTOM_DVE_ROW_BASE + len(dve_ops.OPS)
    op = HandDveOp("SEGMIN3_DO", spec, subdim=True, uops_sha={})
    dve_ops.OPS.append(op)
    dve_ops.CUSTOM_DVE_SPECS[op.name] = spec
    dve_ops._SUB_OPCODE_FOR_NAME[op.name] = row
    _SEG = op
    return op


# mask-column layout in the per-core "msk" input [128, 24]:
# (blk*12 + kri*6 + j), kri: 0->kr=0, 1->kr=2; j: 0 scale_h17(.25m),
# 1 bias_h17, 2 scale_w(2m), 3 bias_w4(-2-2m), 4 scale_h4(.5m).
def _mcol(blk, kr, j):
    return blk * 12 + (0 if kr == 0 else 1) * 6 + j


def _build_nc():  # noqa: C901
    enc = _enc_op()
    seg = _seg_op()
    nc = bacc.Bacc("TRN2", target_bir_lowering=False)
    dpad = nc.dram_tensor("dpad", [INROWS, INCOLS], F32, kind="ExternalInput")
    msk = nc.dram_tensor("msk", [128, 24], F32, kind="ExternalInput")
    out = nc.dram_tensor("out", [18, HALF, W], I32, kind="ExternalOutput")
    out_base = out[:, :, :]
    with tile.TileContext(nc) as tc:
        with (
            tc.tile_pool(name="copies", bufs=1) as cpool,
            tc.tile_pool(name="wplanes", bufs=1) as wpool,
            tc.tile_pool(name="eplanes", bufs=1) as epool,
            tc.tile_pool(name="mtmp", bufs=1) as Epool,
            tc.tile_pool(name="wins", bufs=1) as Kpool,
            tc.tile_pool(name="extr", bufs=1) as ipool,
            tc.tile_pool(name="outs", bufs=1) as opool,
            tc.tile_pool(name="singles", bufs=1) as spool,
        ):
            z = spool.tile([128, W], I32, tag="z")
            nc.gpsimd.memset(z[:, :], 0)

            # two-block shifted copies: block b partition p = dpad row
            # b*BLK1 + p + PAD + dr. The six dr!=0 copies live in two
            # contiguous [128, 3, 2, INCOLS] group tiles so the dr axis is a
            # uniform AP stride (lets taps 1/7 run as SEGMIN3 over dr).
            c0t = cpool.tile([128, 2, INCOLS], F32, tag="c0")
            XH = 320
            CH = XH + PAD  # first column-half covers windows for x < XH
            for b, c_lo, c_hi in ((0, 0, CH), (0, CH, INCOLS), (1, 0, INCOLS)):
                s0 = bass.AP(
                    tensor=dpad[:, :].tensor,
                    offset=(PAD + b * BLK1) * INCOLS + c_lo,
                    ap=[[INCOLS, 128], [1, c_hi - c_lo]],
                )
                nc.sync.dma_start(out=c0t[:, b, c_lo:c_hi], in_=s0)
            groups = {}
            for gname, drs in (("lo", (-6, -4, -2)), ("hi", (2, 4, 6))):
                gt = cpool.tile([128, 3, 2, INCOLS], F32, tag=f"g{gname}")
                for i, dr in enumerate(drs):
                    s1_ = bass.AP(
                        tensor=dpad[:, :].tensor,
                        offset=(PAD + dr) * INCOLS,
                        ap=[[INCOLS, 128], [BLK1 * INCOLS, 2], [1, INCOLS]],
                    )
                    nc.sync.dma_start(out=gt[:, i, :, :], in_=s1_)
                groups[gname] = gt

            def crow(dr):
                """(tile, within-tile offset) of the dr copy row."""
                if dr == 0:
                    return c0t, 0
                g = "lo" if dr < 0 else "hi"
                i = {(-6): 0, -4: 1, -2: 2, 2: 0, 4: 1, 6: 2}[dr]
                return groups[g], i * 2 * INCOLS

            ctr = c0t[:, :, PAD: PAD + W]
            mt = spool.tile([128, 24], F32, tag="msk")
            nc.sync.dma_start(out=mt, in_=msk[:, :])

            # constant-zero channels: off_h of taps 3,4,5; off_w of taps 1,4,7
            for ch in (3, 4, 5, 10, 13, 16):
                for b, p0, nr in ((0, 0, 128), (1, 16, 112)):
                    zdst = bass.AP(
                        tensor=out_base.tensor,
                        offset=out_base.offset + ch * HALF * W + b * (BLK1 + 16) * W,
                        ap=[[W, nr], [1, W]],
                    )
                    nc.sync.dma_start(out=zdst, in_=z[0:nr, :])

            # --- fused window op: one (dr, c0) column window, one block -----
            wcnt = [0]
            WRING = 4

            def w_tile():
                t = wpool.tile([128, 2, W, 3], F32, tag=f"w{wcnt[0] % WRING}")
                wcnt[0] += 1
                return t

            def seg_win(dr, c0, dridx, xsplit=False):
                """Both blocks of one 3-wide column window -> [128,2,W,3]
                running-min stream; page ends at [..., 2]."""
                t = w_tile()
                gt, goff = crow(dr)
                gb = gt[:, 0, :] if dr == 0 else gt[:, 0, 0, :]
                pstride = gb.ap[0][0]
                for b in (0, 1):
                    xr = ((0, XH), (XH, W)) if (xsplit and b == 0) else ((0, W),)
                    for x0, x1 in xr:
                        in0 = bass.AP(
                            tensor=gb.tensor,
                            offset=gb.offset + goff + b * INCOLS + PAD + c0 + x0,
                            ap=[[pstride, 128], [1, x1 - x0], [2, 3]])
                        cb = c0t[:, b, :]
                        in1 = bass.AP(tensor=cb.tensor,
                                      offset=cb.offset + PAD + x0,
                                      ap=[[cb.ap[0][0], 128], [1, x1 - x0], [0, 3]])
                        nc.vector._custom_dve(
                            seg, out=t[:, b, x0:x1, :], in0=in0, in1=in1,
                            s0=_bits_f(15), s1=_bits_f(dridx << 2), imm2=0.0,
                        )
                return t

            # --- taps 1/7 path: Pool subtract + DVE finish (6-bit codes) ----
            ecnt = [0]
            ERING = 7

            def e_tile():
                t = epool.tile([128, 2, W], F32, tag=f"e{ecnt[0] % ERING}")
                ecnt[0] += 1
                return t

            def enc_dve(dr, dc, per_block=False):
                t = e_tile()
                if per_block:
                    for b in (0, 1):
                        nc.vector._custom_dve(
                            enc, out=t[:, b, :],
                            in0=copies[dr][:, b, PAD + dc: PAD + dc + W],
                            in1=copies[0][:, b, PAD: PAD + W],
                            s0=_code_f(dr, dc), s1=_bits_f(63), imm2=0.0,
                        )
                else:
                    nc.vector._custom_dve(
                        enc, out=t[:, :, :],
                        in0=copies[dr][:, :, PAD + dc: PAD + dc + W],
                        in1=ctr, s0=_code_f(dr, dc), s1=_bits_f(63), imm2=0.0,
                    )
                return t

            def sub_pool(dr, dc):
                t = e_tile()
                nc.gpsimd.tensor_tensor(
                    out=t[:, :, :],
                    in0=copies[dr][:, :, PAD + dc: PAD + dc + W],
                    in1=ctr, op=Alu.subtract,
                )
                return t

            def fin_dve(t, dr, dc):
                ti = t.bitcast(I32)
                nc.vector.tensor_scalar(
                    out=ti[:, :, :], in0=ti[:, :, :],
                    scalar1=ABSMASK, scalar2=_code(dr, dc),
                    op0=Alu.bitwise_and, op1=Alu.bitwise_or,
                )
                return t

            mcnt = [0]
            Kcnt = [0]

            def min3(a, b, c, tag):
                t1 = Epool.tile([128, 2, W], F32, tag=f"m{mcnt[0] % 2}")
                mcnt[0] += 1
                nc.vector.tensor_tensor(out=t1[:, :, :], in0=a, in1=b, op=Alu.min)
                t2 = Kpool.tile([128, 2, W], F32, tag=tag)
                nc.vector.tensor_tensor(out=t2[:, :, :], in0=t1[:, :, :],
                                        in1=c, op=Alu.min)
                return t2

            def decode(k, K, seg4):
                """seg4: 4-bit SEGMIN codes (drIdx<<2 | pos). Else 6-bit."""
                kr, kc = divmod(k, 3)
                full = (kr != 1) and (kc != 1)
                Ki = K.bitcast(I32)

                def act_h(dst, src, blk):
                    if seg4:
                        nc.scalar.activation(
                            out=dst, in_=src, func=ActF.Identity,
                            scale=mt[:, _mcol(blk, kr, 4): _mcol(blk, kr, 4) + 1],
                            bias=mt[:, _mcol(blk, kr, 5): _mcol(blk, kr, 5) + 1])
                    else:
                        nc.scalar.activation(
                            out=dst, in_=src, func=ActF.Identity,
                            scale=mt[:, _mcol(blk, kr, 0): _mcol(blk, kr, 0) + 1],
                            bias=mt[:, _mcol(blk, kr, 1): _mcol(blk, kr, 1) + 1])

                def act_w(dst, src, blk):
                    if kr == 1:
                        nc.scalar.activation(out=dst, in_=src, func=ActF.Copy,
                                             scale=2.0, bias=-4.0)
                    else:
                        nc.scalar.activation(
                            out=dst, in_=src, func=ActF.Identity,
                            scale=mt[:, _mcol(blk, kr, 2): _mcol(blk, kr, 2) + 1],
                            bias=mt[:, _mcol(blk, kr, 3): _mcol(blk, kr, 3) + 1])

                hm, wm = (12, 3) if seg4 else (56, 7)
                if full:
                    oo = opool.tile([128, 2, 2, W], I32, tag=f"oo{(k // 2) % 2}")
                    ki_h = ipool.tile([128, 2, W], I32, tag="x56")
                    nc.vector.tensor_scalar(out=ki_h[:, :, :], in0=Ki[:, :, :],
                                            scalar1=hm, scalar2=None,
                                            op0=Alu.bitwise_and)
                    ki_w = ipool.tile([128, 2, W], I32, tag="x7")
                    nc.vector.tensor_scalar(out=ki_w[:, :, :], in0=Ki[:, :, :],
                                            scalar1=wm, scalar2=None,
                                            op0=Alu.bitwise_and)
                    for b, p0, nr in ((0, 0, 128), (1, 16, 112)):
                        act_h(oo[:, 0, b, :], ki_h[:, b, :], b)
                        act_w(oo[:, 1, b, :], ki_w[:, b, :], b)
                        cs = slice(0, 4) if kc == 0 else slice(W - 4, W)
                        for ch_ in (0, 1):
                            nc.scalar.activation(out=oo[:, ch_, b, cs],
                                                 in_=oo[:, ch_, b, cs],
                                                 func=ActF.Copy, scale=0.0,
                                                 bias=-2.0)
                        for ch_i, ch_o in ((0, k), (1, 9 + k)):
                            dst = bass.AP(
                                tensor=out_base.tensor,
                                offset=out_base.offset + ch_o * HALF * W
                                + b * (BLK1 + 16) * W,
                                ap=[[W, nr], [1, W]],
                            )
                            nc.sync.dma_start(out=dst,
                                              in_=oo[p0:p0 + nr, ch_i, b, :])
                else:
                    ob = opool.tile([128, 2, W], I32, tag=f"ob{(k // 2) % 2}")
                    if kc == 1:        # taps 1,7: off_h varies, off_w == 0
                        ki = ipool.tile([128, 2, W], I32, tag="x56")
                        for blk in (0, 1):
                            nc.vector.tensor_scalar(out=ki[:, blk, :],
                                                    in0=Ki[:, blk, :],
                                                    scalar1=hm, scalar2=None,
                                                    op0=Alu.bitwise_and)
                            act_h(ob[:, blk, :], ki[:, blk, :], blk)
                        ch = k
                    else:              # taps 3,5: off_w varies, off_h == 0
                        ki = ipool.tile([128, 2, W], I32, tag="x7")
                        nc.vector.tensor_scalar(out=ki[:, :, :], in0=Ki[:, :, :],
                                                scalar1=wm, scalar2=None,
                                                op0=Alu.bitwise_and)
                        for blk in (0, 1):
                            act_w(ob[:, blk, :], ki[:, blk, :], blk)
                        cs = slice(0, 4) if kc == 0 else slice(W - 4, W)
                        nc.scalar.activation(out=ob[:, :, cs], in_=ob[:, :, cs],
                                             func=ActF.Copy, scale=0.0,
                                             bias=-2.0)
                        ch = 9 + k
                    for b, p0, nr in ((0, 0, 128), (1, 16, 112)):
                        dst = bass.AP(
                            tensor=out_base.tensor,
                            offset=out_base.offset + ch * HALF * W + b * (BLK1 + 16) * W,
                            ap=[[W, nr], [1, W]],
                        )
                        nc.sync.dma_start(out=dst, in_=ob[p0:p0 + nr, b, :])

            # --- corner tap via 3 fused windows --------------------------
            CORNER = {0: ((-6, -4, -2), -6), 2: ((-6, -4, -2), 2),
                      6: ((2, 4, 6), -6), 8: ((2, 4, 6), 2)}

            def corner_tap(k):
                drs, c0 = CORNER[k]
                ws = [seg_win(dr, c0, i) for i, dr in enumerate(drs)]
                pe = [w[:, :, :, 2] for w in ws]
                decode(k, min3(pe[0], pe[1], pe[2], f"K{Kcnt[0] % 2}"), True)
                Kcnt[0] += 1

            def edge_rowtap(k):
                """taps 3/5: one fused window over dc at dr=0; off_w from the
                pos field (K & 3), off_h is a zero channel."""
                c0 = -6 if k == 3 else 2
                kc = 0 if k == 3 else 2
                w = seg_win(0, c0, 0, xsplit=(k == 3))
                wi = w.bitcast(I32)
                ob = opool.tile([128, 2, W], I32, tag=f"ob{(k // 2) % 2}")
                ki = ipool.tile([128, 2, W], I32, tag="xe")
                nc.vector.tensor_scalar(out=ki[:, :, :], in0=wi[:, :, :, 2],
                                        scalar1=3, scalar2=None,
                                        op0=Alu.bitwise_and)
                for blk in (0, 1):
                    nc.scalar.activation(out=ob[:, blk, :], in_=ki[:, blk, :],
                                         func=ActF.Copy, scale=2.0, bias=-4.0)
                cs = slice(0, 4) if kc == 0 else slice(W - 4, W)
                nc.scalar.activation(out=ob[:, :, cs], in_=ob[:, :, cs],
                                     func=ActF.Copy, scale=0.0, bias=-2.0)
                ch = 9 + k
                for b, p0, nr in ((0, 0, 128), (1, 16, 112)):
                    dst = bass.AP(
                        tensor=out_base.tensor,
                        offset=out_base.offset + ch * HALF * W + b * (BLK1 + 16) * W,
                        ap=[[W, nr], [1, W]],
                    )
                    nc.sync.dma_start(out=dst, in_=ob[p0:p0 + nr, b, :])

            def seg_coltap(k):
                """taps 1/7: one fused window over dr (uniform stride inside
                the group tile); off_h from the pos field, off_w is a zero
                channel."""
                kr = 0 if k == 1 else 2
                gt = groups["lo" if k == 1 else "hi"]
                gb = gt[:, 0, 0, :]
                pstride = gb.ap[0][0]
                t = w_tile()
                for b in (0, 1):
                    in0 = bass.AP(tensor=gb.tensor,
                                  offset=gb.offset + b * INCOLS + PAD,
                                  ap=[[pstride, 128], [1, W], [2 * INCOLS, 3]])
                    cb = c0t[:, b, :]
                    in1 = bass.AP(tensor=cb.tensor, offset=cb.offset + PAD,
                                  ap=[[cb.ap[0][0], 128], [1, W], [0, 3]])
                    nc.vector._custom_dve(
                        seg, out=t[:, b, :, :], in0=in0, in1=in1,
                        s0=_bits_f(15), s1=_bits_f(0), imm2=0.0,
                    )
                wi = t.bitcast(I32)
                ob = opool.tile([128, 2, W], I32, tag=f"ob{(k // 2) % 2}")
                ki = ipool.tile([128, 2, W], I32, tag="xe")
                for blk in (0, 1):
                    nc.vector.tensor_scalar(out=ki[:, blk, :],
                                            in0=wi[:, blk, :, 2],
                                            scalar1=3, scalar2=None,
                                            op0=Alu.bitwise_and)
                    nc.scalar.activation(
                        out=ob[:, blk, :], in_=ki[:, blk, :], func=ActF.Identity,
                        scale=mt[:, _mcol(blk, kr, 2): _mcol(blk, kr, 2) + 1],
                        bias=mt[:, _mcol(blk, kr, 3): _mcol(blk, kr, 3) + 1])
                    b, p0, nr = (0, 0, 128) if blk == 0 else (1, 16, 112)
                    dst = bass.AP(
                        tensor=out_base.tensor,
                        offset=out_base.offset + k * HALF * W + b * (BLK1 + 16) * W,
                        ap=[[W, nr], [1, W]],
                    )
                    nc.sync.dma_start(out=dst, in_=ob[p0:p0 + nr, blk, :])

            # --- schedule ------------------------------------------------
            edge_rowtap(3)    # DVE starts on copies[0] immediately
            corner_tap(0)
            seg_coltap(1)
            corner_tap(6)
            corner_tap(2)
            corner_tap(8)
            edge_rowtap(5)
            seg_coltap(7)    # cheapest per-block-pipelined tail last
    nc.compile()
    return nc


_NC = None
LAST_RESULTS = None


def _get_nc():
    global _NC
    if _NC is None:
        _NC = _build_nc()
    return _NC


def _mask_cols(half):
    """[128, 24] per-partition decode scale/bias columns (see _mcol)."""
    m = np.zeros((128, 24), np.float32)
    p = np.arange(128)
    for blk in (0, 1):
        y = half * HALF + blk * BLK1 + p
        for kr in (0, 2):
            ok = (y + 4 * (kr - 1) >= 0) & (y + 4 * (kr - 1) < H)
            mm = ok.astype(np.float32)
            m[:, _mcol(blk, kr, 0)] = 0.25 * mm          # taps 1/7 h scale
            m[:, _mcol(blk, kr, 1)] = -2.0 - (4.0 * kr) * mm  # taps 1/7 h bias
            m[:, _mcol(blk, kr, 2)] = 2.0 * mm           # corner w scale
            m[:, _mcol(blk, kr, 3)] = -2.0 - 2.0 * mm    # corner w bias
            m[:, _mcol(blk, kr, 4)] = 0.5 * mm           # corner h scale
            m[:, _mcol(blk, kr, 5)] = -2.0               # corner h bias
    return m


def kernel(depth):
    global LAST_RESULTS
    depth = np.asarray(depth, dtype=np.float32)
    d = depth[:, 0]                                   # [4, 480, 640]
    dp = np.pad(d, ((0, 0), (PAD, PAD), (PAD, PAD)))  # [4, 492, 652]
    in_maps = []
    for core in range(8):
        b, half = divmod(core, 2)
        sl = np.ascontiguousarray(dp[b, half * HALF: half * HALF + INROWS, :])
        in_maps.append({"dpad": sl, "msk": _mask_cols(half)})
    res = run_bass_kernel_spmd(_get_nc(), in_maps, core_ids=list(range(8)))
    LAST_RESULTS = res
    out = np.zeros((B, 18, H, W), np.int32)
    for core, r in enumerate(res.results):
        b, half = divmod(core, 2)
        out[b, :, half * HALF: (half + 1) * HALF, :] = r["out"]
    return out
